# revision 7
# baseline (speedup 1.0000x reference)
"""Contrastive loss (InfoNCE, labels=arange) Trainium2 Bass kernel.

Problem: x, y [8192, 1024] f32.
  xn = l2norm(x); yn = l2norm(y)
  logits = xn @ yn.T / tau            [8192, 8192]
  loss = -mean(diag(log_softmax(logits)))

Strategy (8 NeuronCores, SPMD):
  - Data-parallel shard x rows: core c gets x[c*1024:(c+1)*1024] plus the
    matching diagonal rows of y; every core streams the full y.
  - Per core: normalize x shard + y (bf16), Gram matmul [1024, 8192] in
    bf16 (fp32 PSUM accum), fused exp+row-sum on ScalarE straight out of
    PSUM (no max subtraction needed: |cos/tau| <= ~14.3, exp is safe in
    fp32), diagonal via elementwise dot in natural layout.
  - Matmul operands need D on the partition axis, so normalized bf16
    tiles bounce through DRAM and come back via the DMA xbar transpose.
  - 1/||row|| computed as exp(-0.5*ln(sumsq)) to stay in the exp/log ACT
    table set (Rsqrt is banned / sqrt would table-thrash).
  - Output per core: per-row loss [128, 8]; host sums and divides by B.

Numerics vs reference (numpy simulation): rel err ~2e-6.
"""

import numpy as np

import concourse.bacc as bacc
import concourse.tile as tile
from concourse import mybir
from concourse.bass_utils import run_bass_kernel_spmd

B = 8192
D = 1024
N_CORES = 8
ROWS = B // N_CORES          # 1024 x-rows per core
MT = ROWS // 128             # 8 m-tiles per core
KT = D // 128                # 8 k-chunks of the contraction dim
YBLK = 1024                  # y rows processed per pipeline block
NYB = B // YBLK              # 8 y blocks
JT = YBLK // 128             # sub-tiles per y block
TAU = 0.07

BF16 = mybir.dt.bfloat16
F32 = mybir.dt.float32
AF = mybir.ActivationFunctionType
ALU = mybir.AluOpType

_compiled = None


def _build():
    nc = bacc.Bacc(
        "TRN2", target_bir_lowering=False, debug=False, num_devices=N_CORES
    )
    xs = nc.dram_tensor("xs", [ROWS, D], F32, kind="ExternalInput").ap()
    y = nc.dram_tensor("y", [B, D], F32, kind="ExternalInput").ap()
    yd = nc.dram_tensor("yd", [ROWS, D], F32, kind="ExternalInput").ap()
    out = nc.dram_tensor("out", [128, MT], F32, kind="ExternalOutput").ap()

    with tile.TileContext(nc) as tc:
        with (
            tc.tile_pool(name="persist", bufs=1) as persist,
            tc.tile_pool(name="xprep", bufs=2) as xprep,
            tc.tile_pool(name="yprep", bufs=2) as yprep,
            tc.tile_pool(name="ynTp", bufs=2) as ynTp,
            tc.tile_pool(name="scr", bufs=3) as scr,
            tc.tile_pool(name="small", bufs=6) as small,
            tc.tile_pool(name="psum", bufs=4, space="PSUM") as psum,
            tc.tile_pool(name="dram", bufs=2, space="DRAM") as dram,
        ):
            xnT = persist.tile([128, KT, ROWS], BF16)      # [d_chunk][k][m]
            sumexp = persist.tile([128, MT, 2 * NYB], F32)
            diag = persist.tile([128, MT], F32)            # diag cosine
            lossb = persist.tile([128, MT], F32)

            def act_rownorm_scale(t, tag):
                """t: [128, D] bf16 natural tile -> t / ||row||, via ACT
                Square+accum (sumsq) and exp(-0.5*ln(ss))."""
                sq = scr.tile([128, D], BF16, tag="sq", name=f"sq_{tag}")
                ss = small.tile([128, 1], F32, tag="ss", name=f"ss_{tag}")
                nc.scalar.activation(out=sq, in_=t, func=AF.Square, accum_out=ss)
                rn = small.tile([128, 1], F32, tag="rn", name=f"rn_{tag}")
                nc.scalar.activation(out=rn, in_=ss, func=AF.Ln)
                nc.scalar.activation(out=rn, in_=rn, func=AF.Exp, scale=-0.5)
                nc.vector.tensor_scalar_mul(out=t, in0=t, scalar1=rn)

            # ---------- x prep: normalize shard, diagonal dot, transpose ----------
            xnd = dram.tile([ROWS, D], BF16, bufs=1)
            for mi in range(MT):
                xb = xprep.tile([128, D], BF16, tag="xb", name=f"xb{mi}")
                nc.gpsimd.dma_start(out=xb, in_=xs[mi * 128:(mi + 1) * 128, :])
                ydb = xprep.tile([128, D], BF16, tag="ydb", name=f"ydb{mi}")
                nc.gpsimd.dma_start(out=ydb, in_=yd[mi * 128:(mi + 1) * 128, :])
                act_rownorm_scale(xb, f"x{mi}")
                act_rownorm_scale(ydb, f"yd{mi}")
                # diag cosine: rowwise dot of normalized tiles (DVE)
                dprod = scr.tile([128, D], BF16, tag="dprod", name=f"dprod{mi}")
                nc.vector.tensor_mul(out=dprod, in0=xb, in1=ydb)
                nc.vector.tensor_reduce(
                    out=diag[:, mi:mi + 1], in_=dprod,
                    axis=mybir.AxisListType.X, op=ALU.add,
                )
                nc.sync.dma_start(out=xnd[mi * 128:(mi + 1) * 128, :], in_=xb)
            for k in range(KT):
                nc.sync.dma_start(
                    out=xnT[:, k:k + 1, :],
                    in_=xnd[:, k * 128:(k + 1) * 128],
                    transpose=True,
                )

            # ---------- y stream: normalize block, transpose, matmul+exp ----------
            for jb in range(NYB):
                ybt = yprep.tile([128, JT, D], BF16)
                ssb = yprep.tile([128, JT], F32, tag="ssb", name=f"ssb{jb}")
                for ji in range(JT):
                    r0 = jb * YBLK + ji * 128
                    nc.gpsimd.dma_start(out=ybt[:, ji, :], in_=y[r0:r0 + 128, :])
                    sq = scr.tile([128, D], BF16, tag="ysq", name=f"ysq{jb}_{ji}")
                    nc.vector.tensor_mul(out=sq, in0=ybt[:, ji, :], in1=ybt[:, ji, :])
                    nc.vector.tensor_reduce(
                        out=ssb[:, ji:ji + 1], in_=sq,
                        axis=mybir.AxisListType.X, op=ALU.add,
                    )
                # batched 1/||row||: one Ln + one Exp for the whole block
                rnb = yprep.tile([128, JT], F32, tag="rnb", name=f"rnb{jb}")
                nc.scalar.activation(out=rnb, in_=ssb, func=AF.Ln)
                nc.scalar.activation(out=rnb, in_=rnb, func=AF.Exp, scale=-0.5)
                for ji in range(JT):
                    nc.vector.tensor_scalar_mul(
                        out=ybt[:, ji, :], in0=ybt[:, ji, :],
                        scalar1=rnb[:, ji:ji + 1],
                    )
                ynd = dram.tile([YBLK, D], BF16)
                nc.sync.dma_start(
                    out=ynd.rearrange("(ji p) d -> p ji d", p=128), in_=ybt
                )
                ynT = ynTp.tile([128, KT, YBLK], BF16)
                for k in range(KT):
                    nc.sync.dma_start(
                        out=ynT[:, k:k + 1, :],
                        in_=ynd[:, k * 128:(k + 1) * 128],
                        transpose=True,
                    )
                for nh in range(YBLK // 512):
                    for mi in range(MT):
                        ps = psum.tile([128, 512], F32)
                        for k in range(KT):
                            nc.tensor.matmul(
                                ps,
                                lhsT=xnT[:, k:k + 1, mi * 128:(mi + 1) * 128],
                                rhs=ynT[:, k:k + 1, nh * 512:(nh + 1) * 512],
                                start=(k == 0),
                                stop=(k == KT - 1),
                            )
                        col = jb * (YBLK // 512) + nh
                        nc.scalar.activation(
                            out=ps, in_=ps, func=AF.Exp, scale=1.0 / TAU,
                            accum_out=sumexp[:, mi, col:col + 1],
                        )

            # ---------- finalize: loss_row = log(sum_exp) - diag/tau ----------
            for mi in range(MT):
                S = small.tile([128, 1], F32, tag="S", name=f"S{mi}")
                nc.vector.tensor_reduce(
                    out=S, in_=sumexp[:, mi:mi + 1, :],
                    axis=mybir.AxisListType.X, op=ALU.add,
                )
                lse = small.tile([128, 1], F32, tag="lse", name=f"lse{mi}")
                nc.scalar.activation(out=lse, in_=S, func=AF.Ln)
                dsc = small.tile([128, 1], F32, tag="dsc", name=f"dsc{mi}")
                nc.vector.tensor_scalar_mul(
                    out=dsc, in0=diag[:, mi:mi + 1], scalar1=1.0 / TAU
                )
                nc.vector.tensor_sub(
                    out=lossb[:, mi:mi + 1], in0=lse, in1=dsc
                )
            nc.sync.dma_start(out=out[:, :], in_=lossb)

    nc.compile()
    return nc


def kernel(x: np.ndarray, y: np.ndarray) -> np.ndarray:
    global _compiled
    if _compiled is None:
        _compiled = _build()
    nc = _compiled

    x = np.ascontiguousarray(x, dtype=np.float32)
    y = np.ascontiguousarray(y, dtype=np.float32)
    in_maps = []
    for c in range(N_CORES):
        sl = slice(c * ROWS, (c + 1) * ROWS)
        in_maps.append({"xs": x[sl], "y": y, "yd": y[sl]})

    res = run_bass_kernel_spmd(nc, in_maps, core_ids=list(range(N_CORES)))
    total = 0.0
    for c in range(N_CORES):
        total += res.results[c]["out"].astype(np.float64).sum()
    return np.float32(total / B)
